# revision 40
# baseline (speedup 1.0000x reference)
"""HCLT probabilistic-circuit kernel for 8 Trainium2 NeuronCores.

Math: the reference collapses algebraically. With
  lp0 + lp1 summed in log space, exp'd, mixed by w_sum, then logsumexp'd,
the whole network is
  out[b] = log( sum_{k,m} w_sum[k] * W0[k,m,x0_b] * W1[k,m,x1_b] )
        = log( A[x0_b, x1_b] ),   A = sum_k w_k * W0[k].T @ W1[k]  (shape [C, C])

Distribution: shard the latent axis k (256) across 8 cores (32 each). Each core
streams its W shard in fp8e4 (sqrt(w_sum) folded into both factors on the host,
per-tensor scaled into fp8 range) and computes the partial
A_c = sum_{k in shard} w0q[k].T @ w1q[k] with DoubleRow fp8 matmuls (2
contraction chunks per instruction). The [256, 256] partial table is DMA'd out
in bf16; the host sums the 8 partials (the unshard of the k-sharded reduction),
applies the inverse scale, gathers the 1024 (x0_b, x1_b) entries and takes the
log.
"""

import sys

import numpy as np

sys.path.insert(0, "/opt/trn_rl_repo")

import ml_dtypes

B, V, M, C = 1024, 2, 256, 256
NCORES = 8
KSH = M // NCORES          # k per core = 32
KM = KSH * M               # flattened contraction rows per core = 8192
NCHUNK = KM // 128         # 64 matmul chunks of 128 rows
# W DMA pieces, in chunks: moderate ramp, 7 pieces (stays under the ~6
# outstanding-DMA ring depth so issues never throttle the stream)
# Small pieces during the DMA stream's slow warm-up ramp (lets the PE start
# early), growing once the stream reaches full rate. The PE drains DoubleRow
# matmuls faster than the DMA delivers, so the chase is DMA-paced throughout;
# this schedule measured best across ~10 variants.
# Six pieces (≤6/ring: no HWDGE ring-depth throttle), small first piece so
# matmuls start during the bandwidth ramp, moderate later pieces so the PE
# (which runs at its slower ~213ns/instr cadence when DMA-paced) never
# accumulates a large post-stream backlog.
PIECES = [2, 8, 12, 16, 18, 8]

_cache = {}


def _build_program():
    import concourse.bacc as bacc
    import concourse.mybir as mybir
    from concourse.tile import TileContext

    f8 = mybir.dt.float8e4
    bf16 = mybir.dt.bfloat16
    f32 = mybir.dt.float32

    nc = bacc.Bacc("TRN2", target_bir_lowering=False)

    x0w = nc.dram_tensor("x0w", [128, NCHUNK * C], f8, kind="ExternalInput")
    x1w = nc.dram_tensor("x1w", [128, NCHUNK * C], f8, kind="ExternalInput")
    aout = nc.dram_tensor("aout", [128, 2 * C], bf16, kind="ExternalOutput")

    with TileContext(nc) as tc:
        with (
            tc.tile_pool(name="wp", bufs=1) as wp,
            tc.tile_pool(name="apool", bufs=1, space="PSUM") as apool,
        ):
            x0sb = wp.tile([128, NCHUNK, C], f8, name="x0sb")
            x1sb = wp.tile([128, NCHUNK, C], f8, name="x1sb")
            asb = wp.tile([128, 2, C], bf16, name="asb")
            # two PSUM tiles in separate banks so the alternating h=0/h=1
            # matmul accumulations don't contend on one bank's write port
            a_ps0 = apool.tile([128, C], f32, name="a_ps0", padded_shape=[128, 512])
            a_ps1 = apool.tile([128, C], f32, name="a_ps1", padded_shape=[128, 512])
            a_ps = [a_ps0, a_ps1]

            j0 = 0
            for p in PIECES:
                sl = slice(j0 * C, (j0 + p) * C)
                nc.sync.dma_start(out=x0sb[:, j0 : j0 + p, :], in_=x0w[:, sl])
                nc.scalar.dma_start(out=x1sb[:, j0 : j0 + p, :], in_=x1w[:, sl])
                j0 += p

            # partial A = sum over 64 chunks of x0q_chunk.T @ x1q_chunk,
            # two chunks per DoubleRow fp8 matmul
            for j in range(0, NCHUNK, 2):
                for h in range(2):
                    nc.tensor.matmul(
                        a_ps[h],
                        lhsT=x0sb[:, j : j + 2, h * 128 : h * 128 + 128],
                        rhs=x1sb[:, j : j + 2, :],
                        start=(j == 0),
                        stop=(j == NCHUNK - 2),
                        perf_mode=mybir.MatmulPerfMode.DoubleRow,
                    )

            # PSUM -> SBUF (f32 -> bf16) on two engines in parallel
            nc.vector.tensor_copy(asb[:, 0, :], a_ps[0])
            nc.scalar.copy(asb[:, 1, :], a_ps[1])
            nc.sync.dma_start(out=aout[:], in_=asb[:])

    nc.compile()
    return nc


def _prep_inputs(x, W, w_sum):
    f8 = ml_dtypes.float8_e4m3
    W = np.asarray(W, dtype=np.float32)
    w_sum = np.asarray(w_sum, dtype=np.float32)

    sq = np.sqrt(w_sum)[:, None, None]
    w0 = W[0] * sq                      # [M(k), M(m), C]
    w1 = W[1] * sq
    s0 = 224.0 / float(w0.max())
    s1 = 224.0 / float(w1.max())
    q0 = (w0 * s0).astype(f8)
    q1 = (w1 * s1).astype(f8)

    in_maps = []
    for c in range(NCORES):
        k0 = c * KSH
        w0c = q0[k0 : k0 + KSH].reshape(KM, C)
        w1c = q1[k0 : k0 + KSH].reshape(KM, C)
        x0wc = np.ascontiguousarray(
            w0c.reshape(NCHUNK, 128, C).transpose(1, 0, 2).reshape(128, NCHUNK * C)
        )
        x1wc = np.ascontiguousarray(
            w1c.reshape(NCHUNK, 128, C).transpose(1, 0, 2).reshape(128, NCHUNK * C)
        )
        in_maps.append({"x0w": x0wc, "x1w": x1wc})
    return in_maps, (s0, s1)


def _run(in_maps, **kwargs):
    from concourse.bass_utils import run_bass_kernel_spmd

    if "nc" not in _cache:
        _cache["nc"] = _build_program()
    return run_bass_kernel_spmd(
        _cache["nc"], in_maps, core_ids=list(range(NCORES)), **kwargs
    )


def _finish(res, scales, x):
    s0, s1 = scales
    x = np.asarray(x)
    a = np.zeros((2, 128, C), dtype=np.float64)
    for r in res.results:
        a += r["aout"].astype(np.float64).reshape(128, 2, C).transpose(1, 0, 2)
    afull = a.reshape(2 * 128, C)
    vals = afull[x[:, 0].astype(np.int64), x[:, 1].astype(np.int64)]
    return (np.log(vals) - np.log(s0) - np.log(s1)).astype(np.float32)


def kernel(x, W, w_sum):
    in_maps, scales = _prep_inputs(x, W, w_sum)
    res = _run(in_maps)
    return _finish(res, scales, x)
